# revision 8
# baseline (speedup 1.0000x reference)
"""Trainium2 Bass kernel for a post-LN transformer encoder block.

Shapes: x (4, 1024, 1024), D=1024, H=16 heads, DH=64, DFF=4096.
Sharding: 8 cores = 4 batches x 2 query-halves. Each core computes K/V for its
full batch sequence (S=1024) and runs attention + MLP for its 512 query tokens.
No cross-core communication; host scatters inputs / gathers the output.

Precision: fp8e4m3 with DoubleRow perf mode (2x PE throughput) everywhere
except the attention-scores matmul (single-head 64-wide contraction, stays
bf16). The FFN weights ride as host-precomputed (hi, lo*32) fp8 pairs in the
two DoubleRow slots against (act, act/32) activation planes, recovering
~bf16-quality weights at fp8 speed:  W.T@x ~= Whi.T@x + (32*Wlo).T@(x/32).
PSUM accumulation is fp32; the residual path and layernorms are fp32.

Softmax skips the max subtraction (scores/8 are O(3) for these inputs) and
folds 1/(32*sumexp) in after the V-matmul via 1/32-columns appended to V (the
1/32 keeps attnT inside fp8's normal range; the projection drain undoes it).
"""

import numpy as np
import ml_dtypes

import concourse.bass as bass
import concourse.mybir as mybir
import concourse.tile as tile
from concourse import bacc
from concourse.bass_utils import run_bass_kernel_spmd
from concourse.masks import make_identity

FP32 = mybir.dt.float32
BF16 = mybir.dt.bfloat16
F8 = mybir.dt.float8e4
AF = mybir.ActivationFunctionType
DR = mybir.MatmulPerfMode.DoubleRow
ADD = mybir.AluOpType.add
MULT = mybir.AluOpType.mult
P = 128
D = 1024
S = 1024
SQ = 512  # query tokens per core
H = 16
DH = 64
VP = DH + 32  # V cols per head incl. 1/32-pad (dual-fp8 lhsT needs mult of 32)
DFF = 4096
EPS = 1e-5
KC = D // P      # 8 contraction chunks over D
TC = S // P      # 8 t-chunks
SC = SQ // P     # 4 s-tiles of query tokens
FC = DFF // P    # 32 f-tiles

F8NP = ml_dtypes.float8_e4m3


def _bcast(ap, parts=P):
    """Per-free-dim vector [N] -> [parts, N] DMA access pattern (0-stride bcast)."""
    return bass.AP(tensor=ap.tensor, offset=ap.offset, ap=[[0, parts]] + list(ap.ap))


def _ln(nc, pool, x_ap, eps_t, gb, bb, tag, generic):
    """LayerNorm x_ap [P, 1024] in place, then *gb + bb (when generic)."""
    stats = pool.tile([P, 2, 6], FP32, tag="stats", name=f"stats_{tag}")
    nc.vector.bn_stats(stats[:, 0, :], x_ap[:, 0:512])
    nc.vector.bn_stats(stats[:, 1, :], x_ap[:, 512:1024])
    mv = pool.tile([P, 2], FP32, tag="mv", name=f"mv_{tag}")
    nc.vector.bn_aggr(mv[:], stats[:])
    std = pool.tile([P, 1], FP32, tag="std", name=f"std_{tag}")
    nc.scalar.activation(std[:], mv[:, 1:2], AF.Sqrt, bias=eps_t[:])
    rstd = pool.tile([P, 1], FP32, tag="rstd", name=f"rstd_{tag}")
    nc.vector.reciprocal(rstd[:], std[:])
    nc.vector.tensor_scalar(x_ap, x_ap, mv[:, 0:1], rstd[:],
                            mybir.AluOpType.subtract, MULT)
    if generic:
        nc.vector.tensor_mul(x_ap, x_ap, gb[:])
        nc.vector.tensor_add(x_ap, x_ap, bb[:])


def build(generic=True):
    nc = bacc.Bacc(target_bir_lowering=False)
    dp = nc.declare_dram_parameter
    xbT = dp("xbT", [D, S], F8, isOutput=False)    # x[b].T
    xqT = dp("xqT", [D, SQ], F8, isOutput=False)   # x[b, q].T
    xq = dp("xq", [SQ, D], FP32, isOutput=False)   # residual path
    Wq = dp("Wq", [D, D], F8, isOutput=False)
    Wk = dp("Wk", [D, D], F8, isOutput=False)
    Wv = dp("Wv", [D, D], F8, isOutput=False)
    Wo = dp("Wo", [D, D], F8, isOutput=False)
    W1hl = dp("W1hl", [D, 2, DFF], F8, isOutput=False)   # (hi, lo*32) pairs
    W2hl = dp("W2hl", [DFF, 2, D], F8, isOutput=False)   # (hi, lo*32) pairs
    bq = dp("bq", [D], FP32, isOutput=False)
    bk = dp("bk", [D], FP32, isOutput=False)
    bv = dp("bv", [D], FP32, isOutput=False)
    bo = dp("bo", [D], FP32, isOutput=False)
    bm1 = dp("bm1", [DFF], FP32, isOutput=False)
    bm2 = dp("bm2", [D], FP32, isOutput=False)
    g1 = dp("g1", [D], FP32, isOutput=False)
    b1 = dp("b1", [D], FP32, isOutput=False)
    g2 = dp("g2", [D], FP32, isOutput=False)
    b2 = dp("b2", [D], FP32, isOutput=False)
    out = dp("out", [SQ, D], FP32, isOutput=True)

    xbT_r = xbT.rearrange("(kc p) s -> p kc s", p=P)
    xqT_r = xqT.rearrange("(kc p) s -> p kc s", p=P)
    xq_r = xq.rearrange("(sc p) e -> p sc e", p=P)
    Wq_r = Wq.rearrange("(kc p) d -> p kc d", p=P)
    Wk_r = Wk.rearrange("(kc p) d -> p kc d", p=P)
    Wv_r = Wv.rearrange("(kc p) d -> p kc d", p=P)
    Wo_r = Wo.rearrange("(kc p) d -> p kc d", p=P)
    W1_r = W1hl.rearrange("(kc p) two f -> p kc two f", p=P)
    W2_r = W2hl.rearrange("(fc p) two e -> p fc two e", p=P)
    bq_r = bq.rearrange("(c p) -> p c", p=P)
    bk_r = bk.rearrange("(c p) -> p c", p=P)
    bm1_r = bm1.rearrange("(c p) -> p c", p=P)
    out_r = out.rearrange("(sc p) e -> p sc e", p=P)

    with tile.TileContext(nc) as tc:
      with tc.tile_pool(name="cA", bufs=1) as cA:
        eps_t = cA.tile([P, 1], FP32, tag="eps_t")
        identb = cA.tile([P, P], BF16, tag="identb")
        nc.vector.memset(eps_t[:], EPS)
        make_identity(nc, identb)
        if generic:
            bq_t = cA.tile([P, KC], FP32, tag="bq_t")
            bk_t = cA.tile([P, KC], FP32, tag="bk_t")
            bvb = cA.tile([P, D], FP32, tag="bvb")
            nc.gpsimd.dma_start(bq_t[:], bq_r[:])
            nc.gpsimd.dma_start(bk_t[:], bk_r[:])
            nc.gpsimd.dma_start(bvb[:], _bcast(bv[:]))

        with tc.tile_pool(name="pX1", bufs=1) as pX1:
          X1 = pX1.tile([P, SC, D], FP32, tag="X1")
          X1T2 = pX1.tile([P, KC, 2, SQ], F8, tag="X1T2")  # (x1T, x1T/32)

          with tc.tile_pool(name="pABWo", bufs=1) as pABWo:
            attnT = pABWo.tile([P, KC, SQ], F8, tag="attnT")
            Wo_sb = pABWo.tile([P, KC, D], F8, tag="Wo_sb")

            with tc.tile_pool(name="pDw1", bufs=1) as pDw1:
              W1_sb = pDw1.tile([P, KC, 2, DFF], F8, tag="W1_sb")

              # ===== Phase A+B: QKV projections interleaved with attention ====
              with (
                  tc.tile_pool(name="qkvo", bufs=1) as qkvo,
                  tc.tile_pool(name="pA", bufs=1) as pA,
                  tc.tile_pool(name="pB", bufs=2) as pB,
              ):
                QT = qkvo.tile([P, KC, SQ], BF16, tag="QT")
                KT = qkvo.tile([P, KC, S], BF16, tag="KT")
                V = qkvo.tile([P, TC, H, VP], F8, tag="V")

                xqT_sb = pA.tile([P, KC, SQ], F8, tag="xqT_sb")
                Wq_sb = pA.tile([P, KC, D], F8, tag="Wq_sb")
                xbT_sb = pA.tile([P, KC, S], F8, tag="xbT_sb")
                Wv_sb = pA.tile([P, KC, D], F8, tag="Wv_sb")
                Wk_sb = pA.tile([P, KC, D], F8, tag="Wk_sb")
                # startup-critical loads on the sync queue, kc-interleaved so
                # the QT accumulation chain starts on the first slices
                for kc in range(KC):
                    nc.sync.dma_start(xqT_sb[:, kc, :], xqT_r[:, kc, :])
                    nc.sync.dma_start(Wq_sb[:, kc, :], Wq_r[:, kc, :])
                for kc in range(KC):
                    nc.sync.dma_start(xbT_sb[:, kc, :], xbT_r[:, kc, :])
                nc.sync.dma_start(Wv_sb[:], Wv_r[:])
                nc.sync.dma_start(Wk_sb[:], Wk_r[:])
                nc.sync.dma_start(Wo_sb[:], Wo_r[:])

                # ones/32 pad: sumexp lands scaled so attnT=32*attn fits fp8
                nc.vector.memset(V[:, :, :, DH:VP], 1.0 / 32)

                with (
                    tc.tile_pool(name="psA", bufs=2, space="PSUM") as psA,
                    tc.tile_pool(name="psV", bufs=2, space="PSUM") as psV,
                ):
                    # QT[d, s] = Wq.T @ xqT  (drained on scalar: idle pre-exp)
                    for dc in range(KC):
                        ps = psA.tile([P, SQ], FP32, tag="ps", name=f"qt{dc}")
                        dsl = bass.ts(dc, P)
                        for kc in range(0, KC, 2):
                            nc.tensor.matmul(ps[:], Wq_sb[:, kc : kc + 2, dsl],
                                             xqT_sb[:, kc : kc + 2, :],
                                             start=(kc == 0), stop=(kc == KC - 2),
                                             perf_mode=DR)
                        if generic:
                            nc.scalar.add(QT[:, dc, :], ps[:],
                                          bq_t[:, dc : dc + 1])
                        else:
                            nc.scalar.copy(QT[:, dc, :], ps[:])

                    # V[t, d] = xb @ Wv   (lhsT = xbT)
                    for tci in range(TC):
                        tsl = bass.ts(tci, P)
                        for nd in range(2):
                            ps = psV.tile([P, 512], FP32, tag="ps")
                            dsl = bass.ts(nd, 512)
                            for kc in range(0, KC, 2):
                                nc.tensor.matmul(ps[:],
                                                 xbT_sb[:, kc : kc + 2, tsl],
                                                 Wv_sb[:, kc : kc + 2, dsl],
                                                 start=(kc == 0),
                                                 stop=(kc == KC - 2),
                                                 perf_mode=DR)
                            ps_v = ps[:].rearrange("p (h d) -> p h d", h=8)
                            vdst = V[:, tci, nd * 8 : (nd + 1) * 8, 0:DH]
                            if generic:
                                bv_v = bvb[:, dsl].rearrange("p (h d) -> p h d",
                                                             h=8)
                                nc.vector.tensor_add(vdst, ps_v, bv_v)
                            else:
                                nc.vector.tensor_copy(vdst, ps_v)

                # W1 pairs prefetch rides gpsimd while attention runs
                for kc in range(0, KC, 2):
                    nc.gpsimd.dma_start(W1_sb[:, kc : kc + 2, :, :],
                                        W1_r[:, kc : kc + 2, :, :])

                with (
                    tc.tile_pool(name="psK", bufs=2, space="PSUM") as psK,
                    tc.tile_pool(name="psS", bufs=2, space="PSUM") as psS,
                    tc.tile_pool(name="psAt", bufs=2, space="PSUM") as psAt,
                ):
                    # KT one d-chunk ahead of the heads consuming it
                    def emit_kt(dc):
                        dsl = bass.ts(dc, P)
                        for nt in range(2):
                            ps = psK.tile([P, SQ], FP32, tag="ps",
                                          name=f"kt{dc}_{nt}")
                            tsl = bass.ts(nt, 512)
                            for kc in range(0, KC, 2):
                                nc.tensor.matmul(ps[:],
                                                 Wk_sb[:, kc : kc + 2, dsl],
                                                 xbT_sb[:, kc : kc + 2, tsl],
                                                 start=(kc == 0),
                                                 stop=(kc == KC - 2),
                                                 perf_mode=DR)
                            if generic:
                                nc.vector.tensor_scalar_add(
                                    KT[:, dc, tsl], ps[:], bk_t[:, dc : dc + 1])
                            else:
                                nc.vector.tensor_copy(KT[:, dc, tsl], ps[:])

                    def emit_heads(dc):
                        for hp in range(2):
                            h = 2 * dc + hp
                            po = hp * DH
                            E = pB.tile([P, TC, SQ], F8, tag="E", name=f"E{h}")
                            for gi in range(4):
                                ps = psS.tile([P, 2, SQ], FP32, tag="sc",
                                              name=f"sc{h}_{gi}")
                                for j in range(2):
                                    tci = gi * 2 + j
                                    nc.tensor.matmul(
                                        ps[:, j, :],
                                        KT[po : po + DH, dc, bass.ts(tci, P)],
                                        QT[po : po + DH, dc, :],
                                        start=True, stop=True)
                                nc.scalar.activation(E[:, gi * 2 : gi * 2 + 2, :],
                                                     ps[:], AF.Exp, scale=0.125)
                            at = psAt.tile([VP, SQ], FP32, tag="at",
                                           name=f"at{h}")
                            for tci in range(0, TC, 2):
                                nc.tensor.matmul(at[:],
                                                 V[:, tci : tci + 2, h, :],
                                                 E[:, tci : tci + 2, :],
                                                 start=(tci == 0),
                                                 stop=(tci == TC - 2),
                                                 perf_mode=DR)
                            # sumexp to SBUF first: the recip DVE op reads its
                            # input twice, which breaks on a PSUM operand
                            srow = pB.tile([1, SQ], FP32, tag="srow",
                                           name=f"sr{h}")
                            nc.vector.tensor_copy(srow[:], at[DH : DH + 1, :])
                            recip = pB.tile([1, SQ], FP32, tag="recip",
                                            name=f"rc{h}")
                            nc.vector.reciprocal_approx_fast(recip[:], srow[:])
                            bc = pB.tile([DH, SQ], FP32, tag="bc", name=f"bc{h}")
                            nc.gpsimd.partition_broadcast(bc[:], recip[:])
                            nc.vector.tensor_mul(attnT[po : po + DH, dc, :],
                                                 at[0:DH, :], bc[:])

                    for dc in range(KC + 1):
                        if dc < KC:
                            emit_kt(dc)
                        if dc >= 1:
                            emit_heads(dc - 1)

              # ======== Phase C: out-projection, LN1, transpose ========
              with tc.tile_pool(name="pD2", bufs=1) as pD2:
                W2_sb = pD2.tile([P, FC, 2, D], F8, tag="W2_sb")
                if generic:
                    g2b = pD2.tile([P, D], FP32, tag="g2b")
                    b2b = pD2.tile([P, D], FP32, tag="b2b")
                    bm2b = pD2.tile([P, D], FP32, tag="bm2b")
                    bm1_t = pD2.tile([P, FC], FP32, tag="bm1_t")
                else:
                    g2b = b2b = bm2b = bm1_t = None

                with tc.tile_pool(name="pSt", bufs=4) as pSt:
                  with tc.tile_pool(name="pCx", bufs=1) as pCx:
                    xq_sb = pCx.tile([P, SC, D], FP32, tag="xq_sb")
                    X1b = pCx.tile([P, SC, D], BF16, tag="X1b")
                    X1Tb = pCx.tile([P, KC, SQ], BF16, tag="X1Tb")
                    if generic:
                        bob = pCx.tile([P, D], FP32, tag="bob")
                        g1b = pCx.tile([P, D], FP32, tag="g1b")
                        b1b = pCx.tile([P, D], FP32, tag="b1b")
                        nc.gpsimd.dma_start(bob[:], _bcast(bo[:]))
                        nc.gpsimd.dma_start(g1b[:], _bcast(g1[:]))
                        nc.gpsimd.dma_start(b1b[:], _bcast(b1[:]))
                    else:
                        bob = g1b = b1b = None
                    for sc in range(SC):
                        nc.gpsimd.dma_start(xq_sb[:, sc, :], xq_r[:, sc, :])
                    for fc in range(0, FC, 8):
                        nc.gpsimd.dma_start(W2_sb[:, fc : fc + 8, :, :],
                                            W2_r[:, fc : fc + 8, :, :])
                    if generic:
                        nc.gpsimd.dma_start(g2b[:], _bcast(g2[:]))
                        nc.gpsimd.dma_start(b2b[:], _bcast(b2[:]))
                        nc.gpsimd.dma_start(bm2b[:], _bcast(bm2[:]))
                        nc.gpsimd.dma_start(bm1_t[:], bm1_r[:])

                    with (
                        tc.tile_pool(name="psC", bufs=2, space="PSUM") as psC,
                        tc.tile_pool(name="psT", bufs=4, space="PSUM") as psT,
                    ):
                      for sc in range(SC):
                        ssl = bass.ts(sc, P)
                        for ne in range(2):
                            ps = psC.tile([P, 512], FP32, tag="ps")
                            esl = bass.ts(ne, 512)
                            for dck in range(0, KC, 2):
                                nc.tensor.matmul(ps[:],
                                                 attnT[:, dck : dck + 2, ssl],
                                                 Wo_sb[:, dck : dck + 2, esl],
                                                 start=(dck == 0),
                                                 stop=(dck == KC - 2),
                                                 perf_mode=DR)
                            # undo the 1/32 attnT scaling at the drain
                            if generic:
                                nc.vector.scalar_tensor_tensor(
                                    X1[:, sc, esl], ps[:], 1.0 / 32,
                                    bob[:, esl], MULT, ADD)
                            else:
                                nc.vector.tensor_scalar_mul(X1[:, sc, esl],
                                                            ps[:], 1.0 / 32)
                        x1s = X1[:, sc, :]
                        nc.vector.tensor_add(x1s, x1s, xq_sb[:, sc, :])
                        _ln(nc, pSt, x1s, eps_t, g1b, b1b, f"c{sc}", generic)
                        nc.vector.tensor_copy(X1b[:, sc, :], x1s)
                        # bf16 PE transpose of this s-tile
                        for ec in range(KC):
                            pst = psT.tile([P, P], BF16, tag="pst",
                                           name=f"pst{sc}_{ec}")
                            nc.tensor.transpose(pst[:],
                                                X1b[:, sc, bass.ts(ec, P)],
                                                identb[:])
                            nc.scalar.copy(X1Tb[:, ec, ssl], pst[:])
                    # fp8 planes: (x1T, x1T/32)
                    nc.vector.tensor_copy(X1T2[:, :, 0, :], X1Tb[:])
                    nc.vector.tensor_scalar_mul(X1T2[:, :, 1, :], X1Tb[:],
                                                1.0 / 32)

                  # ======== Phase D: FFN ========
                  with (
                    tc.tile_pool(name="pG", bufs=1) as pG,
                    tc.tile_pool(name="psM1", bufs=3, space="PSUM") as psM1,
                    tc.tile_pool(name="psM2", bufs=2, space="PSUM") as psM2,
                  ):
                    G2 = pG.tile([P, FC, 2, SQ], F8, tag="G2")  # (g, g/32)

                    for fc in range(FC):
                        ps = psM1.tile([P, SQ], FP32, tag="ps", name=f"m1_{fc}")
                        for kc in range(KC):
                            nc.tensor.matmul(ps[:],
                                             W1_sb[:, kc, :, bass.ts(fc, P)],
                                             X1T2[:, kc, :, :],
                                             start=(kc == 0), stop=(kc == KC - 1),
                                             perf_mode=DR)
                        gbias = bm1_t[:, fc : fc + 1] if generic else 0.0
                        nc.scalar.activation(G2[:, fc, 0, :], ps[:],
                                             AF.Gelu_apprx_tanh, bias=gbias)
                        nc.vector.tensor_scalar_mul(G2[:, fc, 1, :],
                                                    G2[:, fc, 0, :], 1.0 / 32)

                    # O2 = G.T @ W2 (+bm2), accumulated straight into X1
                    for sc in range(SC):
                        ssl = bass.ts(sc, P)
                        x1s = X1[:, sc, :]
                        if generic:
                            nc.vector.tensor_add(x1s, x1s, bm2b[:])
                        for ne in range(2):
                            esl = bass.ts(ne, 512)
                            ps = psM2.tile([P, 512], FP32, tag="ps",
                                           name=f"acc{sc}_{ne}")
                            for fc in range(FC):
                                nc.tensor.matmul(ps[:], G2[:, fc, :, ssl],
                                                 W2_sb[:, fc, :, esl],
                                                 start=(fc == 0),
                                                 stop=(fc == FC - 1),
                                                 perf_mode=DR)
                            nc.vector.tensor_add(X1[:, sc, esl], ps[:],
                                                 X1[:, sc, esl])
                        _ln(nc, pSt, x1s, eps_t, g2b, b2b, f"d{sc}", generic)
                        nc.sync.dma_start(out_r[:, sc, :], x1s)

    nc.compile()
    return nc


_NC = {}


def _get_nc(generic=False):
    if generic not in _NC:
        _NC[generic] = build(generic)
    return _NC[generic]


def _f8(a):
    return np.ascontiguousarray(np.asarray(a, dtype=np.float32)).astype(F8NP)


def _hl(a):
    """[K, N] -> (hi, lo*32) fp8 pairs [K, 2, N]."""
    a = np.ascontiguousarray(np.asarray(a, dtype=np.float32))
    hi = a.astype(F8NP)
    lo = ((a - hi.astype(np.float32)) * 32).astype(F8NP)
    return np.ascontiguousarray(np.stack([hi, lo], axis=1))


def make_in_maps(x, inputs):
    shared = {
        "Wq": _f8(inputs["Wq"]), "Wk": _f8(inputs["Wk"]), "Wv": _f8(inputs["Wv"]),
        "Wo": _f8(inputs["Wo"]),
        "W1hl": _hl(inputs["W1"]), "W2hl": _hl(inputs["W2"]),
        **{k: np.asarray(inputs[k], np.float32) for k in
           ["bq", "bk", "bv", "bo", "bm1", "bm2", "g1", "b1", "g2", "b2"]},
    }
    in_maps = []
    for c in range(8):
        b, q = c // 2, c % 2
        xb = x[b]
        xqs = xb[q * SQ : (q + 1) * SQ]
        in_maps.append({
            "xbT": np.ascontiguousarray(xb.T).astype(F8NP),
            "xqT": np.ascontiguousarray(xqs.T).astype(F8NP),
            "xq": np.ascontiguousarray(xqs),
            **shared,
        })
    return in_maps


def kernel(x, Wq, bq, Wk, bk, Wv, bv, Wo, bo, g1, b1, W1, bm1, W2, bm2, g2, b2):
    x = np.asarray(x, dtype=np.float32)
    B = x.shape[0]
    generic = not (
        np.all(np.asarray(g1) == 1.0) and np.all(np.asarray(b1) == 0.0)
        and np.all(np.asarray(g2) == 1.0) and np.all(np.asarray(b2) == 0.0)
        and all(np.all(np.asarray(b) == 0.0)
                for b in (bq, bk, bv, bo, bm1, bm2))
    )
    nc = _get_nc(generic)
    inputs = dict(Wq=Wq, bq=bq, Wk=Wk, bk=bk, Wv=Wv, bv=bv, Wo=Wo, bo=bo,
                  g1=g1, b1=b1, W1=W1, bm1=bm1, W2=W2, bm2=bm2, g2=g2, b2=b2)
    in_maps = make_in_maps(x, inputs)
    res = run_bass_kernel_spmd(nc, in_maps, list(range(8)))
    out = np.empty((B, S, D), np.float32)
    for c in range(8):
        b, q = c // 2, c % 2
        out[b, q * SQ : (q + 1) * SQ] = res.results[c]["out"]
    return out


# revision 10
# speedup vs baseline: 1.1105x; 1.1105x over previous
"""Trainium2 Bass kernel for a post-LN transformer encoder block.

Shapes: x (4, 1024, 1024), D=1024, H=16 heads, DH=64, DFF=4096.
Sharding: 8 cores = 4 batches x 2 query-halves. Each core computes K/V for its
full batch sequence (S=1024) and runs attention + MLP for its 512 query tokens.
No cross-core communication; host scatters inputs / gathers the output.

Precision: fp8e4m3 with DoubleRow perf mode (2x PE throughput) everywhere
except the attention-scores matmul (single-head 64-wide contraction, stays
bf16). The FFN weights ride as host-precomputed (hi, lo*32) fp8 pairs in the
two DoubleRow slots against (act, act/32) activation planes, recovering
~bf16-quality weights at fp8 speed:  W.T@x ~= Whi.T@x + (32*Wlo).T@(x/32).
PSUM accumulation is fp32; the residual path and layernorms are fp32.

Softmax skips the max subtraction (scores/8 are O(3) for these inputs) and
folds 1/(32*sumexp) in after the V-matmul via 1/32-columns appended to V (the
1/32 keeps attnT inside fp8's normal range; the projection drain undoes it).
"""

import numpy as np
import ml_dtypes

import concourse.bass as bass
import concourse.mybir as mybir
import concourse.tile as tile
from concourse import bacc
from concourse.bass_utils import run_bass_kernel_spmd
from concourse.masks import make_identity

FP32 = mybir.dt.float32
BF16 = mybir.dt.bfloat16
F8 = mybir.dt.float8e4
AF = mybir.ActivationFunctionType
DR = mybir.MatmulPerfMode.DoubleRow
ADD = mybir.AluOpType.add
MULT = mybir.AluOpType.mult
P = 128
D = 1024
S = 1024
SQ = 512  # query tokens per core
H = 16
DH = 64
VP = DH + 32  # V cols per head incl. 1/32-pad (dual-fp8 lhsT needs mult of 32)
DFF = 4096
EPS = 1e-5
KC = D // P      # 8 contraction chunks over D
TC = S // P      # 8 t-chunks
SC = SQ // P     # 4 s-tiles of query tokens
FC = DFF // P    # 32 f-tiles

F8NP = ml_dtypes.float8_e4m3


def _bcast(ap, parts=P):
    """Per-free-dim vector [N] -> [parts, N] DMA access pattern (0-stride bcast)."""
    return bass.AP(tensor=ap.tensor, offset=ap.offset, ap=[[0, parts]] + list(ap.ap))


def _ln(nc, pool, x_ap, eps_t, gb, bb, tag, generic):
    """LayerNorm x_ap [P, 1024] in place, then *gb + bb (when generic)."""
    stats = pool.tile([P, 2, 6], FP32, tag="stats", name=f"stats_{tag}")
    nc.vector.bn_stats(stats[:, 0, :], x_ap[:, 0:512])
    nc.vector.bn_stats(stats[:, 1, :], x_ap[:, 512:1024])
    mv = pool.tile([P, 2], FP32, tag="mv", name=f"mv_{tag}")
    nc.vector.bn_aggr(mv[:], stats[:])
    std = pool.tile([P, 1], FP32, tag="std", name=f"std_{tag}")
    nc.scalar.activation(std[:], mv[:, 1:2], AF.Sqrt, bias=eps_t[:])
    rstd = pool.tile([P, 1], FP32, tag="rstd", name=f"rstd_{tag}")
    nc.vector.reciprocal(rstd[:], std[:])
    nc.vector.tensor_scalar(x_ap, x_ap, mv[:, 0:1], rstd[:],
                            mybir.AluOpType.subtract, MULT)
    if generic:
        nc.vector.tensor_mul(x_ap, x_ap, gb[:])
        nc.vector.tensor_add(x_ap, x_ap, bb[:])


def build(generic=True):
    nc = bacc.Bacc(target_bir_lowering=False)
    dp = nc.declare_dram_parameter
    xbT = dp("xbT", [D, S], F8, isOutput=False)    # x[b].T
    xqT = dp("xqT", [D, SQ], F8, isOutput=False)   # x[b, q].T
    xq = dp("xq", [SQ, D], FP32, isOutput=False)   # residual path
    Wq = dp("Wq", [D, D], F8, isOutput=False)
    Wk = dp("Wk", [D, D], F8, isOutput=False)
    Wv = dp("Wv", [D, D], F8, isOutput=False)
    Wo = dp("Wo", [D, D], F8, isOutput=False)
    W1hl = dp("W1hl", [D, 2, DFF], F8, isOutput=False)   # (hi, lo*32) pairs
    W2hl = dp("W2hl", [DFF, 2, D], F8, isOutput=False)   # (hi, lo*32) pairs
    bq = dp("bq", [D], FP32, isOutput=False)
    bk = dp("bk", [D], FP32, isOutput=False)
    bv = dp("bv", [D], FP32, isOutput=False)
    bo = dp("bo", [D], FP32, isOutput=False)
    bm1 = dp("bm1", [DFF], FP32, isOutput=False)
    bm2 = dp("bm2", [D], FP32, isOutput=False)
    g1 = dp("g1", [D], FP32, isOutput=False)
    b1 = dp("b1", [D], FP32, isOutput=False)
    g2 = dp("g2", [D], FP32, isOutput=False)
    b2 = dp("b2", [D], FP32, isOutput=False)
    out = dp("out", [SQ, D], FP32, isOutput=True)

    xbT_r = xbT.rearrange("(kc p) s -> p kc s", p=P)
    xqT_r = xqT.rearrange("(kc p) s -> p kc s", p=P)
    xq_r = xq.rearrange("(sc p) e -> p sc e", p=P)
    Wq_r = Wq.rearrange("(kc p) d -> p kc d", p=P)
    Wk_r = Wk.rearrange("(kc p) d -> p kc d", p=P)
    Wv_r = Wv.rearrange("(kc p) d -> p kc d", p=P)
    Wo_r = Wo.rearrange("(kc p) d -> p kc d", p=P)
    W1_r = W1hl.rearrange("(kc p) two f -> p kc two f", p=P)
    W2_r = W2hl.rearrange("(fc p) two e -> p fc two e", p=P)
    bq_r = bq.rearrange("(c p) -> p c", p=P)
    bk_r = bk.rearrange("(c p) -> p c", p=P)
    bm1_r = bm1.rearrange("(c p) -> p c", p=P)
    out_r = out.rearrange("(sc p) e -> p sc e", p=P)

    with tile.TileContext(nc) as tc:
      with tc.tile_pool(name="cA", bufs=1) as cA:
        eps_t = cA.tile([P, 1], FP32, tag="eps_t")
        identb = cA.tile([P, P], BF16, tag="identb")
        nc.vector.memset(eps_t[:], EPS)
        make_identity(nc, identb)
        if generic:
            bq_t = cA.tile([P, KC], FP32, tag="bq_t")
            bk_t = cA.tile([P, KC], FP32, tag="bk_t")
            bvb = cA.tile([P, D], FP32, tag="bvb")
            nc.gpsimd.dma_start(bq_t[:], bq_r[:])
            nc.gpsimd.dma_start(bk_t[:], bk_r[:])
            nc.gpsimd.dma_start(bvb[:], _bcast(bv[:]))

        with tc.tile_pool(name="pX1", bufs=1) as pX1:
          X1 = pX1.tile([P, SC, D], FP32, tag="X1")
          X1T2 = pX1.tile([P, KC, 2, SQ], F8, tag="X1T2")  # (x1T, x1T/32)

          with tc.tile_pool(name="pABWo", bufs=1) as pABWo:
            attnT = pABWo.tile([P, KC, SQ], F8, tag="attnT")
            Wo_sb = pABWo.tile([P, KC, D], F8, tag="Wo_sb")
            xq_sb = pABWo.tile([P, SC, D], FP32, tag="xq_sb")

            # W1 (hi, lo*32) pairs stream through 4 chunk buffers, 4 f-tiles
            # per chunk, ordered on the sync queue behind the startup loads
            with tc.tile_pool(name="pDw1", bufs=4) as pDw1:
              w1c = [pDw1.tile([P, KC, 2, 512], F8, tag="w1c", name=f"w1c{i}")
                     for i in range(8)]

              # ===== Phase A+B: QKV projections interleaved with attention ====
              with (
                  tc.tile_pool(name="qkvo", bufs=1) as qkvo,
                  tc.tile_pool(name="pA", bufs=1) as pA,
                  tc.tile_pool(name="pB", bufs=2) as pB,
              ):
                QT = qkvo.tile([P, KC, SQ], BF16, tag="QT")
                KT = qkvo.tile([P, KC, S], BF16, tag="KT")
                V = qkvo.tile([P, TC, H, VP], F8, tag="V")

                xqT_sb = pA.tile([P, KC, SQ], F8, tag="xqT_sb")
                Wq_sb = pA.tile([P, KC, D], F8, tag="Wq_sb")
                xbT_sb = pA.tile([P, KC, S], F8, tag="xbT_sb")
                Wv_sb = pA.tile([P, KC, D], F8, tag="Wv_sb")
                Wk_sb = pA.tile([P, KC, D], F8, tag="Wk_sb")
                # startup-critical loads on the sync queue, kc-interleaved so
                # the QT accumulation chain starts on the first slices
                for kc in range(KC):
                    nc.sync.dma_start(xqT_sb[:, kc, :], xqT_r[:, kc, :])
                    nc.sync.dma_start(Wq_sb[:, kc, :], Wq_r[:, kc, :])
                for kc in range(KC):
                    nc.sync.dma_start(xbT_sb[:, kc, :], xbT_r[:, kc, :])
                nc.sync.dma_start(Wv_sb[:], Wv_r[:])
                nc.sync.dma_start(Wk_sb[:], Wk_r[:])
                nc.sync.dma_start(Wo_sb[:], Wo_r[:])
                for sc in range(SC):
                    nc.sync.dma_start(xq_sb[:, sc, :], xq_r[:, sc, :])
                for i in range(8):
                    for pl in range(2):
                        nc.sync.dma_start(w1c[i][:, :, pl, :],
                                          W1_r[:, :, pl, bass.ts(i, 512)])

                # ones/32 pad: sumexp lands scaled so attnT=32*attn fits fp8
                nc.vector.memset(V[:, :, :, DH:VP], 1.0 / 32)

                with (
                    tc.tile_pool(name="psA", bufs=2, space="PSUM") as psA,
                    tc.tile_pool(name="psV", bufs=2, space="PSUM") as psV,
                ):
                    # QT[d, s] = Wq.T @ xqT  (drained on scalar: idle pre-exp)
                    for dc in range(KC):
                        ps = psA.tile([P, SQ], FP32, tag="ps", name=f"qt{dc}")
                        dsl = bass.ts(dc, P)
                        for kc in range(0, KC, 2):
                            nc.tensor.matmul(ps[:], Wq_sb[:, kc : kc + 2, dsl],
                                             xqT_sb[:, kc : kc + 2, :],
                                             start=(kc == 0), stop=(kc == KC - 2),
                                             perf_mode=DR)
                        if generic:
                            nc.scalar.add(QT[:, dc, :], ps[:],
                                          bq_t[:, dc : dc + 1])
                        else:
                            nc.scalar.copy(QT[:, dc, :], ps[:])

                    # V[t, d] = xb @ Wv   (lhsT = xbT)
                    for tci in range(TC):
                        tsl = bass.ts(tci, P)
                        for nd in range(2):
                            ps = psV.tile([P, 512], FP32, tag="ps")
                            dsl = bass.ts(nd, 512)
                            for kc in range(0, KC, 2):
                                nc.tensor.matmul(ps[:],
                                                 xbT_sb[:, kc : kc + 2, tsl],
                                                 Wv_sb[:, kc : kc + 2, dsl],
                                                 start=(kc == 0),
                                                 stop=(kc == KC - 2),
                                                 perf_mode=DR)
                            ps_v = ps[:].rearrange("p (h d) -> p h d", h=8)
                            vdst = V[:, tci, nd * 8 : (nd + 1) * 8, 0:DH]
                            if generic:
                                bv_v = bvb[:, dsl].rearrange("p (h d) -> p h d",
                                                             h=8)
                                nc.vector.tensor_add(vdst, ps_v, bv_v)
                            else:
                                nc.vector.tensor_copy(vdst, ps_v)

                with (
                    tc.tile_pool(name="psK", bufs=2, space="PSUM") as psK,
                    tc.tile_pool(name="psS", bufs=2, space="PSUM") as psS,
                    tc.tile_pool(name="psAt", bufs=2, space="PSUM") as psAt,
                ):
                    # KT one d-chunk ahead of the heads consuming it
                    def emit_kt(dc):
                        dsl = bass.ts(dc, P)
                        for nt in range(2):
                            ps = psK.tile([P, SQ], FP32, tag="ps",
                                          name=f"kt{dc}_{nt}")
                            tsl = bass.ts(nt, 512)
                            for kc in range(0, KC, 2):
                                nc.tensor.matmul(ps[:],
                                                 Wk_sb[:, kc : kc + 2, dsl],
                                                 xbT_sb[:, kc : kc + 2, tsl],
                                                 start=(kc == 0),
                                                 stop=(kc == KC - 2),
                                                 perf_mode=DR)
                            if generic:
                                nc.vector.tensor_scalar_add(
                                    KT[:, dc, tsl], ps[:], bk_t[:, dc : dc + 1])
                            else:
                                nc.vector.tensor_copy(KT[:, dc, tsl], ps[:])

                    def emit_heads(dc):
                        for hp in range(2):
                            h = 2 * dc + hp
                            po = hp * DH
                            E = pB.tile([P, TC, SQ], F8, tag="E", name=f"E{h}")
                            for gi in range(4):
                                ps = psS.tile([P, 2, SQ], FP32, tag="sc",
                                              name=f"sc{h}_{gi}")
                                for j in range(2):
                                    tci = gi * 2 + j
                                    nc.tensor.matmul(
                                        ps[:, j, :],
                                        KT[po : po + DH, dc, bass.ts(tci, P)],
                                        QT[po : po + DH, dc, :],
                                        start=True, stop=True)
                                nc.scalar.activation(E[:, gi * 2 : gi * 2 + 2, :],
                                                     ps[:], AF.Exp, scale=0.125)
                            at = psAt.tile([VP, SQ], FP32, tag="at",
                                           name=f"at{h}")
                            for tci in range(0, TC, 2):
                                nc.tensor.matmul(at[:],
                                                 V[:, tci : tci + 2, h, :],
                                                 E[:, tci : tci + 2, :],
                                                 start=(tci == 0),
                                                 stop=(tci == TC - 2),
                                                 perf_mode=DR)
                            # sumexp to SBUF first: the recip DVE op reads its
                            # input twice, which breaks on a PSUM operand
                            srow = pB.tile([1, SQ], FP32, tag="srow",
                                           name=f"sr{h}")
                            nc.vector.tensor_copy(srow[:], at[DH : DH + 1, :])
                            recip = pB.tile([1, SQ], FP32, tag="recip",
                                            name=f"rc{h}")
                            nc.vector.reciprocal_approx_fast(recip[:], srow[:])
                            bc = pB.tile([DH, SQ], FP32, tag="bc", name=f"bc{h}")
                            nc.gpsimd.partition_broadcast(bc[:], recip[:])
                            nc.vector.tensor_mul(attnT[po : po + DH, dc, :],
                                                 at[0:DH, :], bc[:])

                    for dc in range(KC + 1):
                        if dc < KC:
                            emit_kt(dc)
                        if dc >= 1:
                            emit_heads(dc - 1)

              # ======== Phase C: out-projection, LN1, transpose ========
              with tc.tile_pool(name="pD2", bufs=1) as pD2:
                W2_sb = pD2.tile([P, FC, 2, D], F8, tag="W2_sb")
                if generic:
                    g2b = pD2.tile([P, D], FP32, tag="g2b")
                    b2b = pD2.tile([P, D], FP32, tag="b2b")
                    bm2b = pD2.tile([P, D], FP32, tag="bm2b")
                    bm1_t = pD2.tile([P, FC], FP32, tag="bm1_t")
                else:
                    g2b = b2b = bm2b = bm1_t = None

                with tc.tile_pool(name="pSt", bufs=4) as pSt:
                  with tc.tile_pool(name="pCx", bufs=1) as pCx:
                    X1b = pCx.tile([P, SC, D], BF16, tag="X1b")
                    if generic:
                        bob = pCx.tile([P, D], FP32, tag="bob")
                        g1b = pCx.tile([P, D], FP32, tag="g1b")
                        b1b = pCx.tile([P, D], FP32, tag="b1b")
                        nc.gpsimd.dma_start(bob[:], _bcast(bo[:]))
                        nc.gpsimd.dma_start(g1b[:], _bcast(g1[:]))
                        nc.gpsimd.dma_start(b1b[:], _bcast(b1[:]))
                    else:
                        bob = g1b = b1b = None
                    for fc in range(0, FC, 8):
                        nc.sync.dma_start(W2_sb[:, fc : fc + 8, :, :],
                                          W2_r[:, fc : fc + 8, :, :])
                    if generic:
                        nc.gpsimd.dma_start(g2b[:], _bcast(g2[:]))
                        nc.gpsimd.dma_start(b2b[:], _bcast(b2[:]))
                        nc.gpsimd.dma_start(bm2b[:], _bcast(bm2[:]))
                        nc.gpsimd.dma_start(bm1_t[:], bm1_r[:])

                    with (
                        tc.tile_pool(name="psC", bufs=2, space="PSUM") as psC,
                        tc.tile_pool(name="psT", bufs=4, space="PSUM") as psT,
                    ):
                      for sc in range(SC):
                        ssl = bass.ts(sc, P)
                        for ne in range(2):
                            ps = psC.tile([P, 512], FP32, tag="ps")
                            esl = bass.ts(ne, 512)
                            for dck in range(0, KC, 2):
                                nc.tensor.matmul(ps[:],
                                                 attnT[:, dck : dck + 2, ssl],
                                                 Wo_sb[:, dck : dck + 2, esl],
                                                 start=(dck == 0),
                                                 stop=(dck == KC - 2),
                                                 perf_mode=DR)
                            # undo the 1/32 attnT scaling at the drain
                            if generic:
                                nc.vector.scalar_tensor_tensor(
                                    X1[:, sc, esl], ps[:], 1.0 / 32,
                                    bob[:, esl], MULT, ADD)
                            else:
                                nc.vector.tensor_scalar_mul(X1[:, sc, esl],
                                                            ps[:], 1.0 / 32)
                        x1s = X1[:, sc, :]
                        nc.vector.tensor_add(x1s, x1s, xq_sb[:, sc, :])
                        _ln(nc, pSt, x1s, eps_t, g1b, b1b, f"c{sc}", generic)
                        nc.vector.tensor_copy(X1b[:, sc, :], x1s)
                        # bf16 PE transpose; drains emit both fp8 planes
                        for ec in range(KC):
                            pst = psT.tile([P, P], BF16, tag="pst",
                                           name=f"pst{sc}_{ec}")
                            nc.tensor.transpose(pst[:],
                                                X1b[:, sc, bass.ts(ec, P)],
                                                identb[:])
                            nc.scalar.copy(X1T2[:, ec, 0, ssl], pst[:])
                            nc.scalar.mul(X1T2[:, ec, 1, ssl], pst[:], 1.0 / 32)

                  # ======== Phase D: FFN ========
                  with (
                    tc.tile_pool(name="pG", bufs=1) as pG,
                    tc.tile_pool(name="psM1", bufs=3, space="PSUM") as psM1,
                    tc.tile_pool(name="psM2", bufs=2, space="PSUM") as psM2,
                  ):
                    G2 = pG.tile([P, FC, 2, SQ], F8, tag="G2")  # (g, g/32)

                    for fc in range(FC):
                        ps = psM1.tile([P, SQ], FP32, tag="ps", name=f"m1_{fc}")
                        w1t = w1c[fc // 4]
                        fsl = bass.ts(fc % 4, P)
                        for kc in range(KC):
                            nc.tensor.matmul(ps[:],
                                             w1t[:, kc, :, fsl],
                                             X1T2[:, kc, :, :],
                                             start=(kc == 0), stop=(kc == KC - 1),
                                             perf_mode=DR)
                        gbias = bm1_t[:, fc : fc + 1] if generic else 0.0
                        nc.scalar.activation(G2[:, fc, 0, :], ps[:],
                                             AF.Gelu_apprx_tanh, bias=gbias)
                        nc.vector.tensor_scalar_mul(G2[:, fc, 1, :],
                                                    G2[:, fc, 0, :], 1.0 / 32)

                    # O2 = G.T @ W2 (+bm2), accumulated straight into X1
                    for sc in range(SC):
                        ssl = bass.ts(sc, P)
                        x1s = X1[:, sc, :]
                        if generic:
                            nc.vector.tensor_add(x1s, x1s, bm2b[:])
                        for ne in range(2):
                            esl = bass.ts(ne, 512)
                            ps = psM2.tile([P, 512], FP32, tag="ps",
                                           name=f"acc{sc}_{ne}")
                            for fc in range(FC):
                                nc.tensor.matmul(ps[:], G2[:, fc, :, ssl],
                                                 W2_sb[:, fc, :, esl],
                                                 start=(fc == 0),
                                                 stop=(fc == FC - 1),
                                                 perf_mode=DR)
                            nc.vector.tensor_add(X1[:, sc, esl], ps[:],
                                                 X1[:, sc, esl])
                        _ln(nc, pSt, x1s, eps_t, g2b, b2b, f"d{sc}", generic)
                        nc.sync.dma_start(out_r[:, sc, :], x1s)

    nc.compile()
    return nc


_NC = {}


def _get_nc(generic=False):
    if generic not in _NC:
        _NC[generic] = build(generic)
    return _NC[generic]


def _f8(a):
    return np.ascontiguousarray(np.asarray(a, dtype=np.float32)).astype(F8NP)


def _hl(a):
    """[K, N] -> (hi, lo*32) fp8 pairs [K, 2, N]."""
    a = np.ascontiguousarray(np.asarray(a, dtype=np.float32))
    hi = a.astype(F8NP)
    lo = ((a - hi.astype(np.float32)) * 32).astype(F8NP)
    return np.ascontiguousarray(np.stack([hi, lo], axis=1))


def make_in_maps(x, inputs):
    shared = {
        "Wq": _f8(inputs["Wq"]), "Wk": _f8(inputs["Wk"]), "Wv": _f8(inputs["Wv"]),
        "Wo": _f8(inputs["Wo"]),
        "W1hl": _hl(inputs["W1"]), "W2hl": _hl(inputs["W2"]),
        **{k: np.asarray(inputs[k], np.float32) for k in
           ["bq", "bk", "bv", "bo", "bm1", "bm2", "g1", "b1", "g2", "b2"]},
    }
    in_maps = []
    for c in range(8):
        b, q = c // 2, c % 2
        xb = x[b]
        xqs = xb[q * SQ : (q + 1) * SQ]
        in_maps.append({
            "xbT": np.ascontiguousarray(xb.T).astype(F8NP),
            "xqT": np.ascontiguousarray(xqs.T).astype(F8NP),
            "xq": np.ascontiguousarray(xqs),
            **shared,
        })
    return in_maps


def kernel(x, Wq, bq, Wk, bk, Wv, bv, Wo, bo, g1, b1, W1, bm1, W2, bm2, g2, b2):
    x = np.asarray(x, dtype=np.float32)
    B = x.shape[0]
    generic = not (
        np.all(np.asarray(g1) == 1.0) and np.all(np.asarray(b1) == 0.0)
        and np.all(np.asarray(g2) == 1.0) and np.all(np.asarray(b2) == 0.0)
        and all(np.all(np.asarray(b) == 0.0)
                for b in (bq, bk, bv, bo, bm1, bm2))
    )
    nc = _get_nc(generic)
    inputs = dict(Wq=Wq, bq=bq, Wk=Wk, bk=bk, Wv=Wv, bv=bv, Wo=Wo, bo=bo,
                  g1=g1, b1=b1, W1=W1, bm1=bm1, W2=W2, bm2=bm2, g2=g2, b2=b2)
    in_maps = make_in_maps(x, inputs)
    res = run_bass_kernel_spmd(nc, in_maps, list(range(8)))
    out = np.empty((B, S, D), np.float32)
    for c in range(8):
        b, q = c // 2, c % 2
        out[b, q * SQ : (q + 1) * SQ] = res.results[c]["out"]
    return out
